# revision 2
# baseline (speedup 1.0000x reference)
"""AFT-full attention kernel for Trainium2, 8 NeuronCores, data-parallel over batch.

Problem (per reference):
    q = x @ Wq.T + bq ; k = x @ Wk.T + bk ; v = x @ Wv.T + bv
    ek = exp(k); eb = exp(pos_bias)
    num = einsum('ij,bjd->bid', eb, ek*v); den = einsum('ij,bjd->bid', eb, ek)
    out = sigmoid(q) * num / den

Shapes: x [32, 1024, 512], W* [512, 512], pos_bias [1024, 1024].

Strategy: batch-data-parallel, 4 batches per core, no collectives.
Per core: bf16 compute. x / W / exp(pos_bias) are cast to bf16 and round-tripped
through internal DRAM so the transposed operands (d- or j- on partitions) can be
loaded with 2-byte DMA-transpose.
"""

import sys

sys.path.insert(0, "/opt/trn_rl_repo")

import numpy as np

P = 128
D = 512  # d_model
N = 1024  # sequence length
BS = 32
CORES = 8
BPC = BS // CORES  # batches per core
NT = N // P  # 8 n-tiles per batch
ROWS = BPC * N  # 4096 rows of x per core

_CACHE = {}


def _build(kin):
    """Build + compile the per-core graph. kin: contraction size of the
    projection (512 normally; 640 when biases are folded in via augmentation)."""
    import concourse.tile as tile
    from concourse import bacc, mybir
    from contextlib import ExitStack

    f32 = mybir.dt.float32
    bf16 = mybir.dt.bfloat16
    AF = mybir.ActivationFunctionType

    dkt = kin // P  # k-tiles for projections

    nc = bacc.Bacc("TRN2", target_bir_lowering=False, debug=False, num_devices=CORES)

    x_ext = nc.dram_tensor("x", [ROWS, kin], f32, kind="ExternalInput")
    w_ext = [
        nc.dram_tensor(nm, [D, kin], f32, kind="ExternalInput")
        for nm in ("Wq", "Wk", "Wv")
    ]
    pb_ext = nc.dram_tensor("pos_bias", [N, N], f32, kind="ExternalInput")
    out_ext = nc.dram_tensor("out", [ROWS, D], f32, kind="ExternalOutput")

    with tile.TileContext(nc) as tc, ExitStack() as ctx:
        dram = ctx.enter_context(tc.tile_pool(name="dram", bufs=1, space="DRAM"))
        prep = ctx.enter_context(tc.tile_pool(name="prep", bufs=3))
        pbp = ctx.enter_context(tc.tile_pool(name="pbp", bufs=2))
        res = ctx.enter_context(tc.tile_pool(name="res", bufs=1))
        xtp = ctx.enter_context(tc.tile_pool(name="xtp", bufs=2))
        ekp = ctx.enter_context(tc.tile_pool(name="ekp", bufs=2))
        sqp = ctx.enter_context(tc.tile_pool(name="sqp", bufs=2))
        tmp = ctx.enter_context(tc.tile_pool(name="tmp", bufs=3))
        outp = ctx.enter_context(tc.tile_pool(name="outp", bufs=3))
        psum = ctx.enter_context(tc.tile_pool(name="psum", bufs=2, space="PSUM"))

        # internal bf16 DRAM round-trip buffers
        x16 = dram.tile([ROWS, kin], bf16)
        w16 = [dram.tile([D, kin], bf16, name=f"w16_{i}") for i in range(3)]
        eb16 = dram.tile([N, N], bf16)

        # ---- Phase 0a: eb = exp(pos_bias) -> bf16 DRAM ----
        for i in range(NT):
            pb_t = pbp.tile([P, N], f32, tag="pb_ld")
            nc.sync.dma_start(pb_t[:], pb_ext[i * P : (i + 1) * P, :])
            eb_t = pbp.tile([P, N], bf16, tag="pb_ex")
            nc.scalar.activation(eb_t[:], pb_t[:], AF.Exp)
            nc.sync.dma_start(eb16[i * P : (i + 1) * P, :], eb_t[:])

        # ---- Phase 0b: W -> bf16 DRAM ----
        for wi in range(3):
            for r in range(D // P):
                w_t = prep.tile([P, kin], f32, tag="w_ld")
                nc.sync.dma_start(w_t[:], w_ext[wi][r * P : (r + 1) * P, :])
                w_c = prep.tile([P, kin], bf16, tag="w_cast")
                nc.vector.tensor_copy(w_c[:], w_t[:])
                nc.sync.dma_start(w16[wi][r * P : (r + 1) * P, :], w_c[:])

        # ---- Phase 0c: x -> bf16 DRAM ----
        for r in range(ROWS // P):
            x_t = prep.tile([P, kin], f32, tag="x_ld")
            nc.sync.dma_start(x_t[:], x_ext[r * P : (r + 1) * P, :])
            x_c = prep.tile([P, kin], bf16, tag="x_cast")
            nc.vector.tensor_copy(x_c[:], x_t[:])
            nc.sync.dma_start(x16[r * P : (r + 1) * P, :], x_c[:])

        # ---- Phase 0d: resident transposed operands ----
        # EBT[j] [128, 1024] : EBT[j][p, i] = eb[i, j*128+p]   (lhsT for AFT)
        ebt = []
        for j in range(NT):
            t = res.tile([P, N], bf16, name=f"ebt{j}")
            nc.sync.dma_start(t[:], eb16[:, j * P : (j + 1) * P], transpose=True)
            ebt.append(t)
        # WT[w][dt] [128, 512] : WT[w][dt][p, dout] = W[w][dout, dt*128+p]  (rhs)
        wt = []
        for wi in range(3):
            per_w = []
            for dt in range(dkt):
                t = res.tile([P, D], bf16, name=f"wt{wi}_{dt}")
                nc.sync.dma_start(
                    t[:], w16[wi][:, dt * P : (dt + 1) * P], transpose=True
                )
                per_w.append(t)
            wt.append(per_w)

        # ---- per batch ----
        for b in range(BPC):
            r0 = b * N
            # xT[dt] [128, 1024] : xT[dt][p, n] = x16[r0+n, dt*128+p]   (lhsT)
            xt = []
            for dt in range(dkt):
                t = xtp.tile([P, N], bf16, tag=f"xt{dt}")
                nc.sync.dma_start(
                    t[:], x16[r0 : r0 + N, dt * P : (dt + 1) * P], transpose=True
                )
                xt.append(t)

            ek = [None] * NT
            ekv = [None] * NT
            sigq = [None] * NT
            # projections: per n-tile, accumulate q/k/v over dkt k-tiles
            for ni in range(NT):
                q_ps = psum.tile([P, D], f32, tag="ps_a")
                k_ps = psum.tile([P, D], f32, tag="ps_b")
                v_ps = psum.tile([P, D], f32, tag="ps_c")
                nsl = slice(ni * P, (ni + 1) * P)
                for dt in range(dkt):
                    st, sp = dt == 0, dt == dkt - 1
                    nc.tensor.matmul(q_ps[:], xt[dt][:, nsl], wt[0][dt][:], start=st, stop=sp)
                    nc.tensor.matmul(k_ps[:], xt[dt][:, nsl], wt[1][dt][:], start=st, stop=sp)
                    nc.tensor.matmul(v_ps[:], xt[dt][:, nsl], wt[2][dt][:], start=st, stop=sp)
                sigq[ni] = sqp.tile([P, D], bf16, tag=f"sq{ni}", name=f"sq{ni}")
                nc.scalar.activation(sigq[ni][:], q_ps[:], AF.Sigmoid)
                ek[ni] = ekp.tile([P, D], bf16, tag=f"ek{ni}", name=f"ek{ni}")
                nc.scalar.activation(ek[ni][:], k_ps[:], AF.Exp)
                ekv[ni] = ekp.tile([P, D], bf16, tag=f"ekv{ni}", name=f"ekv{ni}")
                nc.vector.tensor_mul(ekv[ni][:], ek[ni][:], v_ps[:])

            # AFT: num/den per i-tile, accumulate over j-tiles
            for ii in range(NT):
                num_ps = psum.tile([P, D], f32, tag="ps_a")
                den_ps = psum.tile([P, D], f32, tag="ps_b")
                isl = slice(ii * P, (ii + 1) * P)
                for j in range(NT):
                    st, sp = j == 0, j == NT - 1
                    nc.tensor.matmul(num_ps[:], ebt[j][:, isl], ekv[j][:], start=st, stop=sp)
                    nc.tensor.matmul(den_ps[:], ebt[j][:, isl], ek[j][:], start=st, stop=sp)
                recip = tmp.tile([P, D], f32, tag="recip")
                nc.vector.reciprocal_approx_fast(recip[:], den_ps[:])
                t1 = tmp.tile([P, D], f32, tag="t1")
                nc.vector.tensor_mul(t1[:], recip[:], num_ps[:])
                o_t = outp.tile([P, D], f32, tag="ot")
                nc.vector.tensor_mul(o_t[:], t1[:], sigq[ii][:])
                nc.sync.dma_start(out_ext[r0 + ii * P : r0 + (ii + 1) * P, :], o_t[:])

    nc.compile()
    return nc


def _get_nc(kin):
    if kin not in _CACHE:
        _CACHE[kin] = _build(kin)
    return _CACHE[kin]


def kernel(x, Wq, bq, Wk, bk, Wv, bv, pos_bias):
    from concourse.bass_utils import run_bass_kernel_spmd

    x = np.ascontiguousarray(x, dtype=np.float32)
    no_bias = not (np.any(bq) or np.any(bk) or np.any(bv))
    if no_bias:
        kin = D
        xk = x.reshape(BS * N, D)
        wqk, wkk, wvk = Wq, Wk, Wv
    else:
        # fold biases in by augmenting the contraction dim to 640
        kin = D + P
        xk = np.zeros((BS * N, kin), np.float32)
        xk[:, :D] = x.reshape(BS * N, D)
        xk[:, D] = 1.0

        def aug(W, b):
            Wa = np.zeros((D, kin), np.float32)
            Wa[:, :D] = W
            Wa[:, D] = b
            return Wa

        wqk, wkk, wvk = aug(Wq, bq), aug(Wk, bk), aug(Wv, bv)

    nc = _get_nc(kin)
    in_maps = []
    for c in range(CORES):
        in_maps.append(
            {
                "x": np.ascontiguousarray(xk[c * ROWS : (c + 1) * ROWS]),
                "Wq": np.ascontiguousarray(wqk, dtype=np.float32),
                "Wk": np.ascontiguousarray(wkk, dtype=np.float32),
                "Wv": np.ascontiguousarray(wvk, dtype=np.float32),
                "pos_bias": np.ascontiguousarray(pos_bias, dtype=np.float32),
            }
        )
    res = run_bass_kernel_spmd(nc, in_maps, core_ids=list(range(CORES)))
    out = np.concatenate([res.results[c]["out"] for c in range(CORES)], axis=0)
    return out.reshape(BS, N, D)


# revision 3
# speedup vs baseline: 1.0890x; 1.0890x over previous
"""AFT-full attention kernel for Trainium2, 8 NeuronCores, data-parallel over batch.

Problem (per reference):
    q = x @ Wq.T + bq ; k = x @ Wk.T + bk ; v = x @ Wv.T + bv
    ek = exp(k); eb = exp(pos_bias)
    num = einsum('ij,bjd->bid', eb, ek*v); den = einsum('ij,bjd->bid', eb, ek)
    out = sigmoid(q) * num / den

Shapes: x [32, 1024, 512], W* [512, 512], pos_bias [1024, 1024].

Strategy: batch-data-parallel, 4 batches per core, no collectives.
bf16 tensor-engine compute. x / W / exp(pos_bias) are cast to bf16 and
round-tripped through internal DRAM (per-batch granularity so reads only
depend on their own writes) so the transposed operands (d- or j- on
partitions) can be loaded with 2-byte DMA-transpose.

sigmoid is computed via the ScalarE Exp table only (avoids LUT reloads):
    out = num / (den * (1 + exp(-q)))
"""

import sys

sys.path.insert(0, "/opt/trn_rl_repo")

import numpy as np

P = 128
D = 512  # d_model
N = 1024  # sequence length
BS = 32
CORES = 8
BPC = BS // CORES  # batches per core
NT = N // P  # 8 n-tiles per batch
ROWS = BPC * N  # 4096 rows of x per core

_CACHE = {}


def _build(kin):
    """Build + compile the per-core graph. kin: contraction size of the
    projection (512 normally; 640 when biases are folded in via augmentation)."""
    import concourse.tile as tile
    from concourse import bacc, mybir
    from contextlib import ExitStack

    f32 = mybir.dt.float32
    bf16 = mybir.dt.bfloat16
    AF = mybir.ActivationFunctionType
    ALU = mybir.AluOpType

    dkt = kin // P  # k-tiles for projections

    nc = bacc.Bacc("TRN2", target_bir_lowering=False, debug=False, num_devices=CORES)

    x_ext = nc.dram_tensor("x", [ROWS, kin], f32, kind="ExternalInput")
    w_ext = [
        nc.dram_tensor(nm, [D, kin], f32, kind="ExternalInput")
        for nm in ("Wq", "Wk", "Wv")
    ]
    pb_ext = nc.dram_tensor("pos_bias", [N, N], f32, kind="ExternalInput")
    out_ext = nc.dram_tensor("out", [ROWS, D], f32, kind="ExternalOutput")

    with tile.TileContext(nc) as tc, ExitStack() as ctx:
        dram = ctx.enter_context(tc.tile_pool(name="dram", bufs=1, space="DRAM"))
        prep = ctx.enter_context(tc.tile_pool(name="prep", bufs=4))
        pbp = ctx.enter_context(tc.tile_pool(name="pbp", bufs=2))
        res = ctx.enter_context(tc.tile_pool(name="res", bufs=1))
        xtp = ctx.enter_context(tc.tile_pool(name="xtp", bufs=2))
        ekp = ctx.enter_context(tc.tile_pool(name="ekp", bufs=2))
        eqp = ctx.enter_context(tc.tile_pool(name="eqp", bufs=2))
        tmp = ctx.enter_context(tc.tile_pool(name="tmp", bufs=3))
        outp = ctx.enter_context(tc.tile_pool(name="outp", bufs=3))
        psum = ctx.enter_context(tc.tile_pool(name="psum", bufs=2, space="PSUM"))

        # internal bf16 DRAM round-trip buffers (per batch / per weight so
        # transposed reads only depend on their own writes)
        x16 = [dram.tile([N, kin], bf16, name=f"x16_{b}") for b in range(BPC)]
        w16 = [dram.tile([D, kin], bf16, name=f"w16_{i}") for i in range(3)]
        eb16 = dram.tile([N, N], bf16)

        def cast_x_batch(b):
            """x[batch b] f32 -> bf16 DRAM (casts on gpsimd, which is idle)."""
            for r in range(N // P):
                x_t = prep.tile([P, kin], f32, tag="x_ld", name=f"xld{b}_{r}")
                nc.sync.dma_start(x_t[:], x_ext[b * N + r * P : b * N + (r + 1) * P, :])
                x_c = prep.tile([P, kin], bf16, tag="x_cast", name=f"xc{b}_{r}")
                nc.gpsimd.tensor_copy(x_c[:], x_t[:])
                nc.sync.dma_start(x16[b][r * P : (r + 1) * P, :], x_c[:])

        def load_xt_batch(b):
            """transposed lhsT tiles for batch b: xT[dt][p, n] = x16[b][n, dt*128+p]"""
            xt = []
            for dt in range(dkt):
                t = xtp.tile([P, N], bf16, tag=f"xt{dt}", name=f"xt{b}_{dt}")
                nc.sync.dma_start(
                    t[:], x16[b][:, dt * P : (dt + 1) * P], transpose=True
                )
                xt.append(t)
            return xt

        # ---- prep: x batch 0 (feeds first projections) ----
        cast_x_batch(0)

        # ---- prep: W -> bf16 DRAM (casts on gpsimd) ----
        for wi in range(3):
            for r in range(D // P):
                w_t = prep.tile([P, kin], f32, tag="w_ld", name=f"wld{wi}_{r}")
                nc.sync.dma_start(w_t[:], w_ext[wi][r * P : (r + 1) * P, :])
                w_c = prep.tile([P, kin], bf16, tag="w_cast", name=f"wc{wi}_{r}")
                nc.gpsimd.tensor_copy(w_c[:], w_t[:])
                nc.sync.dma_start(w16[wi][r * P : (r + 1) * P, :], w_c[:])

        # ---- prep: eb = exp(pos_bias) -> bf16 DRAM (ACT, Exp only) ----
        for i in range(NT):
            pb_t = pbp.tile([P, N], f32, tag="pb_ld", name=f"pbld{i}")
            nc.sync.dma_start(pb_t[:], pb_ext[i * P : (i + 1) * P, :])
            eb_t = pbp.tile([P, N], bf16, tag="pb_ex", name=f"pbex{i}")
            nc.scalar.activation(eb_t[:], pb_t[:], AF.Exp)
            nc.sync.dma_start(eb16[i * P : (i + 1) * P, :], eb_t[:])

        # ---- resident transposed operands ----
        # WT[w][dt] [128, 512]: WT[w][dt][p, dout] = W[w][dout, dt*128+p]  (rhs)
        wt = []
        for wi in range(3):
            per_w = []
            for dt in range(dkt):
                t = res.tile([P, D], bf16, name=f"wt{wi}_{dt}")
                nc.sync.dma_start(
                    t[:], w16[wi][:, dt * P : (dt + 1) * P], transpose=True
                )
                per_w.append(t)
            wt.append(per_w)
        # EBT[j] [128, 1024]: EBT[j][p, i] = eb[i, j*128+p]   (lhsT for AFT)
        ebt = []
        for j in range(NT):
            t = res.tile([P, N], bf16, name=f"ebt{j}")
            nc.sync.dma_start(t[:], eb16[:, j * P : (j + 1) * P], transpose=True)
            ebt.append(t)

        xt = load_xt_batch(0)

        # ---- per batch ----
        for b in range(BPC):
            if b + 1 < BPC:
                cast_x_batch(b + 1)  # overlap next batch's prep with compute

            ek = [None] * NT
            ekv = [None] * NT
            eq = [None] * NT
            # projections: per n-tile, accumulate q/k/v over dkt k-tiles
            for ni in range(NT):
                q_ps = psum.tile([P, D], f32, tag="ps_a", name=f"qps{b}_{ni}")
                k_ps = psum.tile([P, D], f32, tag="ps_b", name=f"kps{b}_{ni}")
                v_ps = psum.tile([P, D], f32, tag="ps_c", name=f"vps{b}_{ni}")
                nsl = slice(ni * P, (ni + 1) * P)
                for dt in range(dkt):
                    st, sp = dt == 0, dt == dkt - 1
                    nc.tensor.matmul(q_ps[:], xt[dt][:, nsl], wt[0][dt][:], start=st, stop=sp)
                    nc.tensor.matmul(k_ps[:], xt[dt][:, nsl], wt[1][dt][:], start=st, stop=sp)
                    nc.tensor.matmul(v_ps[:], xt[dt][:, nsl], wt[2][dt][:], start=st, stop=sp)
                # eq = exp(-q)  (sigmoid via Exp table only)
                eq[ni] = eqp.tile([P, D], bf16, tag=f"eq{ni}", name=f"eq{b}_{ni}")
                nc.scalar.activation(eq[ni][:], q_ps[:], AF.Exp, scale=-1.0)
                ek[ni] = ekp.tile([P, D], bf16, tag=f"ek{ni}", name=f"ek{b}_{ni}")
                nc.scalar.activation(ek[ni][:], k_ps[:], AF.Exp)
                ekv[ni] = ekp.tile([P, D], bf16, tag=f"ekv{ni}", name=f"ekv{b}_{ni}")
                nc.vector.tensor_mul(ekv[ni][:], ek[ni][:], v_ps[:])

            if b + 1 < BPC:
                xt = load_xt_batch(b + 1)

            # AFT: num/den per i-tile, accumulate over j-tiles
            r0 = b * N
            for ii in range(NT):
                num_ps = psum.tile([P, D], f32, tag="ps_a", name=f"nps{b}_{ii}")
                den_ps = psum.tile([P, D], f32, tag="ps_b", name=f"dps{b}_{ii}")
                isl = slice(ii * P, (ii + 1) * P)
                for j in range(NT):
                    st, sp = j == 0, j == NT - 1
                    nc.tensor.matmul(num_ps[:], ebt[j][:, isl], ekv[j][:], start=st, stop=sp)
                    nc.tensor.matmul(den_ps[:], ebt[j][:, isl], ek[j][:], start=st, stop=sp)
                # den2 = (eq + 1) * den ;  out = num / den2
                den2 = tmp.tile([P, D], f32, tag="den2", name=f"den2_{b}_{ii}")
                nc.vector.scalar_tensor_tensor(
                    den2[:], eq[ii][:], 1.0, den_ps[:], ALU.add, ALU.mult
                )
                recip = tmp.tile([P, D], f32, tag="recip", name=f"recip{b}_{ii}")
                nc.vector.reciprocal_approx_fast(recip[:], den2[:])
                o_t = outp.tile([P, D], f32, tag="ot", name=f"ot{b}_{ii}")
                nc.vector.tensor_mul(o_t[:], recip[:], num_ps[:])
                nc.sync.dma_start(out_ext[r0 + ii * P : r0 + (ii + 1) * P, :], o_t[:])

    nc.compile()
    return nc


def _get_nc(kin):
    if kin not in _CACHE:
        _CACHE[kin] = _build(kin)
    return _CACHE[kin]


def kernel(x, Wq, bq, Wk, bk, Wv, bv, pos_bias):
    from concourse.bass_utils import run_bass_kernel_spmd

    x = np.ascontiguousarray(x, dtype=np.float32)
    no_bias = not (np.any(bq) or np.any(bk) or np.any(bv))
    if no_bias:
        kin = D
        xk = x.reshape(BS * N, D)
        wqk, wkk, wvk = Wq, Wk, Wv
    else:
        # fold biases in by augmenting the contraction dim to 640
        kin = D + P
        xk = np.zeros((BS * N, kin), np.float32)
        xk[:, :D] = x.reshape(BS * N, D)
        xk[:, D] = 1.0

        def aug(W, b):
            Wa = np.zeros((D, kin), np.float32)
            Wa[:, :D] = W
            Wa[:, D] = b
            return Wa

        wqk, wkk, wvk = aug(Wq, bq), aug(Wk, bk), aug(Wv, bv)

    nc = _get_nc(kin)
    in_maps = []
    for c in range(CORES):
        in_maps.append(
            {
                "x": np.ascontiguousarray(xk[c * ROWS : (c + 1) * ROWS]),
                "Wq": np.ascontiguousarray(wqk, dtype=np.float32),
                "Wk": np.ascontiguousarray(wkk, dtype=np.float32),
                "Wv": np.ascontiguousarray(wvk, dtype=np.float32),
                "pos_bias": np.ascontiguousarray(pos_bias, dtype=np.float32),
            }
        )
    res = run_bass_kernel_spmd(nc, in_maps, core_ids=list(range(CORES)))
    out = np.concatenate([res.results[c]["out"] for c in range(CORES)], axis=0)
    return out.reshape(BS, N, D)
